# revision 1
# baseline (speedup 1.0000x reference)
"""MoE SwiGLU experts on 8 TRN2 cores - one-level Strassen on BOTH stages.

Same expert-parallel host routing as the plain kernel, but:
  - Device capacity is capped at C = 1024 (pairs beyond that per expert
    - tens out of 8192 - are computed on the HOST in f32). This makes
    C2 = C/2 <= 512, so every Strassen M_i tile is a single
    [P, <=512] PSUM bank and each PE accumulation group is 8 matmuls of
    N = C2 (~1.7us) - the same PSUM-recycle lead profile as the proven
    plain kernel (avoids the suspected short-lead PE wait stall).
  - Stage 1 ([2048x2048] @ [2048xC]) uses one Strassen level over the
    2x2 split (gate/up rows x hidden halves x column halves): 7
    multiplies instead of 8 (-12.5% PE cycles for the stage). The 7
    stationary combos (wS) and 7 moving combos (xV) are precomputed on
    the host; the device runs the multiplies, copies M_i tiles
    PSUM -> SBUF (ACT), combines them with 8 DVE adds per row block
    (f32), then the usual silu * up into fp16 act.
  - Stage 2 ([2048x1024] @ [1024xC]) is the plain blocked matmul.

Matmul operands fp16; M-combines in f32; output f32.
"""

import numpy as np

import concourse.bass as bass
import concourse.mybir as mybir
from concourse.bass_utils import run_bass_kernel_spmd

F32 = mybir.dt.float32
F16 = mybir.dt.float16
NP_IN_DT = np.float16

E = 8
H = 2048
I = 1024
TOKS = 4096
TOPK = 2
P = 128
NCH1 = 8       # contraction chunks per Strassen multiply (1024 / 128)
RB = 8         # row blocks of each 1024-row Strassen product
NCH_I = I // P
NBLK_HT = H // P

NWS = 2        # ws_sb staging depth
NPM = 4        # ps_m rotation depth
NPSY = 3
NOUT = 3

C_CAP = 1024   # device capacity cap; overflow pairs computed on host


def _t_tiles(C, gran=16):
    assert C % gran == 0 and C >= 128
    n = -(-C // 512)
    base = -(-(C // n) // gran) * gran
    sizes = [base] * (n - 1) + [C - base * (n - 1)]
    tiles = []
    t0 = 0
    for tn in sizes:
        assert 0 < tn <= 512
        tiles.append((t0, tn))
        t0 += tn
    return tiles


def build_nc(C, tiles, repeat=1):
    assert C % 16 == 0 and C <= 2 * 512
    C2 = C // 2
    IH = I // 2                    # 512: intermediate half (stage-2 contraction)
    NC2 = IH // P                  # 4 chunks per stage-2 Strassen multiply

    S1G = RB * 7                   # stage-1 M groups (8 matmuls each, N=C2)
    S2G = RB * 7                   # stage-2 M groups (4 matmuls each, N=C2)
    PE_TOT = S1G + S2G
    CP_TOT = S1G + S2G             # ACT copies both stages
    DVE_RB = 10
    DVE_S1 = RB * DVE_RB           # 80
    NV2 = 5                        # device-built moving combos for stage 2
    DVE_RBO = 8                    # y-assembly ops per output row block
    DVE_TOT = DVE_S1 + NV2 + RB * DVE_RBO   # 149
    ACT1_TOT = RB * 2
    NPS = 8                        # one global PSUM rotation across stages
    # combo slots in v2_sb for i in {0,2,3,5,6}; i=1 (B11) and i=4 (B22)
    # are plain act_sb slices
    V2_SLOT = {0: 0, 2: 1, 3: 2, 5: 3, 6: 4}
    V2_RANK = {0: 1, 2: 2, 3: 3, 5: 4, 6: 5}   # DVE combo completion order

    nc = bass.Bass("TRN2", target_bir_lowering=False, debug=False, num_devices=E)

    xV = nc.dram_tensor("xV", [7, NCH1, P, C2], F16, kind="ExternalInput").ap()
    wS = nc.dram_tensor("wS", [RB, 7, P, NCH1 * P], F16, kind="ExternalInput").ap()
    wS2 = nc.dram_tensor("wS2", [RB, 7, P, IH], F16, kind="ExternalInput").ap()
    yT = nc.dram_tensor("yT", [NBLK_HT, P, C], F32, kind="ExternalOutput").ap()

    xv_sb = nc.alloc_sbuf_tensor("xv_sb", [P, 7, NCH1, C2], F16).ap()
    ws_sb = [nc.alloc_sbuf_tensor(f"ws_sb{b}", [P, 7, NCH1, P], F16).ap()
             for b in range(NWS)]
    w2_sb = [nc.alloc_sbuf_tensor(f"w2_sb{b}", [P, 7, IH], F16).ap()
             for b in range(2)]
    m_sb = [nc.alloc_sbuf_tensor(f"m_sb{b}", [P, 7, C2], F32).ap()
            for b in range(2)]
    cmb = {n: nc.alloc_sbuf_tensor(f"cmb_{n}", [P, C2], F32).ap()
           for n in ("ga", "gb", "g11", "g12", "u21", "ua", "ub", "u22")}
    sg = [nc.alloc_sbuf_tensor(f"sg{b}", [P, C2], F32).ap() for b in range(2)]
    act_sb = nc.alloc_sbuf_tensor("act_sb", [P, NCH_I, C], F16).ap()
    v2_sb = nc.alloc_sbuf_tensor("v2_sb", [P, NV2, NC2, C2], F16).ap()
    NOUT4 = 4
    out_sb = [nc.alloc_sbuf_tensor(f"out_sb{b}", [P, 512], F32).ap()
              for b in range(NOUT4)]

    ps = [nc.alloc_psum_tensor(f"ps{b}", [P, 512], F32).ap()
          for b in range(NPS)]

    import contextlib
    with contextlib.ExitStack() as ctx:
        block = ctx.enter_context(nc.Block())
        dma_xv = [ctx.enter_context(nc.semaphore(f"dma_xv{i}")) for i in range(7)]
        dma_ws = [ctx.enter_context(nc.semaphore(f"dma_ws{r}")) for r in range(RB)]
        dma_w2 = ctx.enter_context(nc.semaphore("dma_w2"))
        dma_ob = [ctx.enter_context(nc.semaphore(f"dma_ob{b}"))
                  for b in range(NOUT4)]
        pe_sem = ctx.enter_context(nc.semaphore("pe_sem"))
        cp_sem = ctx.enter_context(nc.semaphore("cp_sem"))
        act1 = ctx.enter_context(nc.semaphore("act1"))
        dve = ctx.enter_context(nc.semaphore("dve"))

        @block.sync
        def _(sync):
            for it in range(repeat):
                if it > 0:
                    # xv_sb / ws_sb free once prior stage 1 fully drains
                    sync.wait_ge(pe_sem, (it - 1) * PE_TOT + S1G)
                for i2 in range(7):
                    sync.dma_start(ws_sb[0][:, i2],
                                   wS[0, i2]).then_inc(dma_ws[0], 16)
                for i in range(7):
                    for c in range(NCH1):
                        sync.dma_start(xv_sb[:, i, c, :],
                                       xV[i, c]).then_inc(dma_xv[i], 16)
                for rb in range(1, RB):
                    if rb >= NWS:
                        sync.wait_ge(pe_sem, it * PE_TOT + (rb - NWS + 1) * 7)
                    for i2 in range(7):
                        sync.dma_start(ws_sb[rb % NWS][:, i2],
                                       wS[rb, i2]).then_inc(dma_ws[rb], 16)
                # stage-2 stationary combos, streamed 2-deep like ws_sb.
                # All loads share the sync HWDGE queue (in-order), so one
                # total-count semaphore is safe.
                for rbo in range(RB):
                    if rbo >= 2:
                        sync.wait_ge(pe_sem,
                                     it * PE_TOT + S1G + (rbo - 1) * 7)
                    elif it > 0:
                        sync.wait_ge(pe_sem,
                                     (it - 1) * PE_TOT + S1G + (rbo + 7) * 7)
                    for i2 in range(7):
                        sync.dma_start(w2_sb[rbo % 2][:, i2],
                                       wS2[rbo, i2]).then_inc(dma_w2, 16)

        @block.tensor
        def _(tensor):
            for it in range(repeat):
                g1 = 0
                for rb in range(RB):
                    tensor.wait_ge(dma_ws[rb], 112 * (it + 1))
                    for i in range(7):
                        if rb == 0:
                            tensor.wait_ge(dma_xv[i], 128 * (it + 1))
                        cp_tgt = it * CP_TOT + g1 - (NPS - 1)
                        if cp_tgt > 0:
                            tensor.wait_ge(cp_sem, cp_tgt)
                        for c in range(NCH1):
                            mm = tensor.matmul(
                                ps[g1 % NPS][:, :C2],
                                ws_sb[rb % NWS][:, i, c, :],
                                xv_sb[:, i, c, :],
                                start=(c == 0), stop=(c == NCH1 - 1),
                            )
                        mm.then_inc(pe_sem, 1)
                        g1 += 1
                for rbo in range(RB):
                    tensor.wait_ge(dma_w2, it * 896 + (rbo + 1) * 112)
                    for i in range(7):
                        G = S1G + rbo * 7 + i
                        if rbo == 0:
                            tensor.wait_ge(dve, it * DVE_TOT + DVE_S1
                                           + V2_RANK.get(i, 0))
                        cp_tgt = it * CP_TOT + G - (NPS - 1)
                        if cp_tgt > 0:
                            tensor.wait_ge(cp_sem, cp_tgt)
                        for c in range(NC2):
                            if i == 1:
                                mov = act_sb[:, c, 0:C2]
                            elif i == 4:
                                mov = act_sb[:, NC2 + c, C2:C]
                            else:
                                mov = v2_sb[:, V2_SLOT[i], c, :]
                            mm = tensor.matmul(
                                ps[G % NPS][:, :C2],
                                w2_sb[rbo % 2][:, i, c * P:(c + 1) * P],
                                mov,
                                start=(c == 0), stop=(c == NC2 - 1),
                            )
                        mm.then_inc(pe_sem, 1)

        @block.scalar
        def _(scalar):
            def silu_pair(it, r):
                scalar.wait_ge(dve, it * DVE_TOT + r * DVE_RB + 3)
                scalar.activation(sg[0], cmb["g11"],
                                  mybir.ActivationFunctionType.Silu,
                                  ).then_inc(act1, 1)
                scalar.wait_ge(dve, it * DVE_TOT + r * DVE_RB + 4)
                scalar.activation(sg[1], cmb["g12"],
                                  mybir.ActivationFunctionType.Silu,
                                  ).then_inc(act1, 1)

            def y_dmas(it, r):
                # DVE y-assembly op counts within rbo=r block: Y11 done at
                # +3, Y12 +4, Y21 +5, Y22 +8 (bufs 0..3)
                base = it * DVE_TOT + DVE_S1 + NV2 + r * DVE_RBO
                for b, (done, ht, col0) in enumerate(
                        [(3, r, 0), (4, r, C2), (5, RB + r, 0),
                         (8, RB + r, C2)]):
                    scalar.wait_ge(dve, base + done)
                    scalar.dma_start(yT[ht][:, col0:col0 + C2],
                                     out_sb[b][:, :C2]
                                     ).then_inc(dma_ob[b], 16)

            for it in range(repeat):
                g1 = 0
                for rb in range(RB):
                    # m_sb[rb%2] free: stage-1 DVE of rb-2 done this iter,
                    # or (rb<2) the prior iter's y-assembly of rbo=rb+6
                    if rb >= 2:
                        scalar.wait_ge(dve, it * DVE_TOT + (rb - 1) * DVE_RB)
                    elif it > 0:
                        scalar.wait_ge(dve, (it - 1) * DVE_TOT + DVE_S1 + NV2
                                       + (rb + 7) * DVE_RBO)
                    for i in range(7):
                        scalar.wait_ge(pe_sem, it * PE_TOT + g1 + 1)
                        scalar.copy(m_sb[rb % 2][:, i, :],
                                    ps[g1 % NPS][:, :C2]).then_inc(cp_sem, 1)
                        g1 += 1
                    if rb >= 1:
                        silu_pair(it, rb - 1)
                silu_pair(it, RB - 1)
                for rbo in range(RB):
                    # m_sb[rbo%2] free: stage-1 DVE of rb=rbo+6 (rbo<2) or
                    # y-assembly of rbo-2 done
                    if rbo >= 2:
                        scalar.wait_ge(dve, it * DVE_TOT + DVE_S1 + NV2
                                       + (rbo - 1) * DVE_RBO)
                    else:
                        scalar.wait_ge(dve, it * DVE_TOT + (rbo + 7) * DVE_RB)
                    for i in range(7):
                        G = S1G + rbo * 7 + i
                        scalar.wait_ge(pe_sem, it * PE_TOT + G + 1)
                        scalar.copy(m_sb[rbo % 2][:, i, :],
                                    ps[G % NPS][:, :C2]).then_inc(cp_sem, 1)
                    if rbo >= 1:
                        y_dmas(it, rbo - 1)
                y_dmas(it, RB - 1)

        @block.vector
        def _(vector):
            for it in range(repeat):
                for rb in range(RB):
                    if rb == 0:
                        vector.wait_ge(pe_sem, it * PE_TOT)  # act_sb free
                    vector.wait_ge(cp_sem, it * CP_TOT + (rb + 1) * 7)
                    mb = lambda i: m_sb[rb % 2][:, i, :]
                    v = vector
                    v.tensor_add(cmb["ga"], mb(0), mb(3)).then_inc(dve, 1)
                    v.tensor_sub(cmb["gb"], mb(6), mb(4)).then_inc(dve, 1)
                    if rb >= 1:
                        v.wait_ge(act1, it * ACT1_TOT + (rb - 1) * 2 + 1)
                    elif it > 0:
                        v.wait_ge(act1, it * ACT1_TOT)
                    v.tensor_add(cmb["g11"], cmb["ga"], cmb["gb"]).then_inc(dve, 1)
                    if rb >= 1:
                        v.wait_ge(act1, it * ACT1_TOT + (rb - 1) * 2 + 2)
                    v.tensor_add(cmb["g12"], mb(2), mb(4)).then_inc(dve, 1)
                    v.tensor_add(cmb["u21"], mb(1), mb(3)).then_inc(dve, 1)
                    v.tensor_sub(cmb["ua"], mb(2), mb(1)).then_inc(dve, 1)
                    v.tensor_add(cmb["ub"], mb(0), mb(5)).then_inc(dve, 1)
                    v.tensor_add(cmb["u22"], cmb["ua"], cmb["ub"]).then_inc(dve, 1)
                    v.wait_ge(act1, it * ACT1_TOT + rb * 2 + 1)
                    v.tensor_mul(act_sb[:, rb, 0:C2], sg[0],
                                 cmb["u21"]).then_inc(dve, 1)
                    v.wait_ge(act1, it * ACT1_TOT + rb * 2 + 2)
                    v.tensor_mul(act_sb[:, rb, C2:C], sg[1],
                                 cmb["u22"]).then_inc(dve, 1)
                # stage-2 moving combos over act halves (B11 = rows 0:512
                # left cols, B22 = rows 512: right cols, etc.)
                v = vector
                B11 = act_sb[:, 0:NC2, 0:C2]
                B12 = act_sb[:, 0:NC2, C2:C]
                B21 = act_sb[:, NC2:2 * NC2, 0:C2]
                B22 = act_sb[:, NC2:2 * NC2, C2:C]
                v.tensor_add(v2_sb[:, 0], B11, B22).then_inc(dve, 1)
                v.tensor_sub(v2_sb[:, 1], B12, B22).then_inc(dve, 1)
                v.tensor_sub(v2_sb[:, 2], B21, B11).then_inc(dve, 1)
                v.tensor_add(v2_sb[:, 3], B11, B12).then_inc(dve, 1)
                v.tensor_add(v2_sb[:, 4], B21, B22).then_inc(dve, 1)
                # y-assembly: Y11=M1+M4-M5+M7, Y12=M3+M5, Y21=M2+M4,
                # Y22=M1-M2+M3+M6 into out_sb[0..3]
                for rbo in range(RB):
                    vector.wait_ge(cp_sem, it * CP_TOT + S1G + (rbo + 1) * 7)
                    mb = lambda i: m_sb[rbo % 2][:, i, :]
                    stc = it * RB + rbo
                    v.tensor_add(cmb["ga"], mb(0), mb(3)).then_inc(dve, 1)
                    v.tensor_sub(cmb["gb"], mb(6), mb(4)).then_inc(dve, 1)
                    if stc > 0:
                        v.wait_ge(dma_ob[0], 16 * stc)
                    v.tensor_add(out_sb[0][:, :C2], cmb["ga"],
                                 cmb["gb"]).then_inc(dve, 1)
                    if stc > 0:
                        v.wait_ge(dma_ob[1], 16 * stc)
                    v.tensor_add(out_sb[1][:, :C2], mb(2), mb(4)).then_inc(dve, 1)
                    if stc > 0:
                        v.wait_ge(dma_ob[2], 16 * stc)
                    v.tensor_add(out_sb[2][:, :C2], mb(1), mb(3)).then_inc(dve, 1)
                    v.tensor_sub(cmb["ua"], mb(2), mb(1)).then_inc(dve, 1)
                    v.tensor_add(cmb["ub"], mb(0), mb(5)).then_inc(dve, 1)
                    if stc > 0:
                        v.wait_ge(dma_ob[3], 16 * stc)
                    v.tensor_add(out_sb[3][:, :C2], cmb["ua"],
                                 cmb["ub"]).then_inc(dve, 1)

    return nc


_NC_CACHE = {}


def _get_nc(C, tiles, repeat=1):
    key = (C, tuple(tiles), repeat)
    if key not in _NC_CACHE:
        _NC_CACHE[key] = build_nc(C, tiles, repeat)
    return _NC_CACHE[key]


def _route(top_k_index):
    """Per-expert (token, k) lists and device capacity (capped at C_CAP)."""
    idx = np.asarray(top_k_index)
    tok_t = [[] for _ in range(E)]
    tok_k = [[] for _ in range(E)]
    for k in range(TOPK):
        col = idx[:, k].astype(np.int64)
        for e in range(E):
            ts = np.nonzero(col == e)[0]
            tok_t[e].append(ts)
            tok_k[e].append(np.full(ts.shape, k, np.int64))
    tok_t = [np.concatenate(v) for v in tok_t]
    tok_k = [np.concatenate(v) for v in tok_k]
    counts = np.array([len(v) for v in tok_t])
    cmax = max(int(counts.max()), 256)
    C = min(C_CAP, ((cmax + 15) // 16) * 16)
    return tok_t, tok_k, C


def _pack_pe_lhsT(A):
    R, K = A.shape
    return (A.reshape(R // P, P, K // P, P)
             .transpose(0, 3, 2, 1)
             .reshape(R // P, P, K))


def _make_in_maps(hidden_states, gate_up_proj, down_proj, tok_t, C):
    """Host routing + Strassen operand packing (first C pairs per expert)."""
    C2 = C // 2
    hidden = np.asarray(hidden_states, np.float32)
    in_maps = []
    for e in range(E):
        n_e = min(len(tok_t[e]), C)
        X = np.zeros((H, C), np.float32)
        if n_e:
            X[:, :n_e] = hidden[tok_t[e][:n_e]].T
        B11 = X[:I, :C2]
        B12 = X[:I, C2:]
        B21 = X[I:, :C2]
        B22 = X[I:, C2:]
        V = np.stack([B11 + B22, B11, B12 - B22, B21 - B11,
                      B22, B11 + B12, B21 + B22])
        xVe = np.ascontiguousarray(V.reshape(7, NCH1, P, C2).astype(NP_IN_DT))

        A = np.asarray(gate_up_proj[e], np.float32)
        A11 = A[:I, :I]
        A12 = A[:I, I:]
        A21 = A[I:, :I]
        A22 = A[I:, I:]
        S = np.stack([A11 + A22, A21 + A22, A11, A22,
                      A11 + A12, A21 - A11, A12 - A22])
        wSe = np.empty((RB, 7, P, I), NP_IN_DT)
        for i in range(7):
            wSe[:, i] = _pack_pe_lhsT(S[i]).astype(NP_IN_DT)

        D = np.asarray(down_proj[e], np.float32)
        IH = I // 2
        D11 = D[:I, :IH]
        D12 = D[:I, IH:]
        D21 = D[I:, :IH]
        D22 = D[I:, IH:]
        S2 = np.stack([D11 + D22, D21 + D22, D11, D22,
                       D11 + D12, D21 - D11, D12 - D22])
        wS2e = np.empty((RB, 7, P, IH), NP_IN_DT)
        for i in range(7):
            wS2e[:, i] = _pack_pe_lhsT(S2[i]).astype(NP_IN_DT)

        in_maps.append({"xV": xVe, "wS": np.ascontiguousarray(wSe),
                        "wS2": np.ascontiguousarray(wS2e)})
    return in_maps


def _host_overflow(hidden, gate_up_proj, down_proj, tok_t, tok_k, C, y_pair):
    """Compute pairs beyond the device capacity C on the host (f32)."""
    for e in range(E):
        n_e = len(tok_t[e])
        if n_e <= C:
            continue
        ids = tok_t[e][C:]
        kds = tok_k[e][C:]
        Xo = hidden[ids]                                   # [n, H]
        gu = Xo @ np.asarray(gate_up_proj[e], np.float32).T  # [n, 2I]
        g, u = gu[:, :I], gu[:, I:]
        act = (g / (1.0 + np.exp(-g))) * u
        y = act @ np.asarray(down_proj[e], np.float32).T     # [n, H]
        y_pair[ids, kds] = y


def kernel(hidden_states, top_k_index, top_k_weights, gate_up_proj, down_proj):
    hidden_states = np.asarray(hidden_states, np.float32)
    top_k_weights = np.asarray(top_k_weights, np.float32)

    tok_t, tok_k, C = _route(top_k_index)
    tiles = _t_tiles(C)
    nc = _get_nc(C, tiles)

    in_maps = _make_in_maps(hidden_states, gate_up_proj, down_proj, tok_t, C)
    res = run_bass_kernel_spmd(nc, in_maps, core_ids=list(range(E)))

    y_pair = np.zeros((TOKS, TOPK, H), np.float32)
    for e in range(E):
        n_e = min(len(tok_t[e]), C)
        if n_e == 0:
            continue
        yT = res.results[e]["yT"]                    # [16, 128, C] f32
        y_e = yT.transpose(2, 0, 1).reshape(C, H)[:n_e]
        y_pair[tok_t[e][:n_e], tok_k[e][:n_e]] = y_e
    _host_overflow(hidden_states, gate_up_proj, down_proj,
                   tok_t, tok_k, C, y_pair)
    out = np.einsum("tkh,tk->th", y_pair, top_k_weights).astype(np.float32)
    return out

